# revision 1
# baseline (speedup 1.0000x reference)
"""3x3 valid conv (single channel) on 8 TRN2 NeuronCores.

Strategy: shard X row-wise (512 output rows/core). Per core, five row strips:
four full strips load 128 input rows each (rows 126s..126s+127, re-reading the
2-row halo from HBM) and produce 126 output rows via 3 banded matmuls per
512-col PSUM tile; a mini-strip loads rows 504..513 (10 rows) and produces the
remaining 8 output rows with K=10/M=8 matmuls, so no SBUF->SBUF reuse copy is
needed. X and the bands are declared float32r end-to-end (TF32-like matmul,
~1 row/cycle at the ramped PE p-state) so no cast/rounding pass exists.

Scheduling: loads ride the SP HWDGE ring in chunks (320K first so the first
matmul starts ~11us in); stores are 2MB on the ACT ring so loads can't
head-of-line-block them. PSUM drains (tensor_scalar add bias) ride the vector
engine during the HBM-paced mid-run; for the endgame (strip 3 + mini-strip)
they alternate vector/scalar so a single engine's ~740ns/tile drain rate
can't gate the PE's ~700ns/tile supply. The mini-strip runs between strips 2
and 3 so its port-limited 8-partition store (~7us) hides under mid-run HBM
traffic on the SP ring, and strip 3's last stage is stored in 1M/512K/512K
pieces with the final piece on the emptiest path to minimize the tail.
"""

import sys

sys.path.insert(0, "/opt/trn_rl_repo")

import numpy as np
from concourse import bass, mybir
from concourse.bass_utils import run_bass_kernel_spmd
from concourse.tile import TileContext

F32 = mybir.dt.float32
F32R = mybir.dt.float32r

H, WIDTH = 4096, 8192
KH, KW = 3, 3
OH, OW = H - KH + 1, WIDTH - KW + 1
N_CORES = 8
RPC = H // N_CORES          # 512 output rows produced per core
IN_ROWS = RPC + KH - 1      # 514 input rows per core (2-row halo)
N_COL_TILES = 16            # 15 x 512 + 1 x 510 = 8190


def _split_multi_waits(nc, max_waits=1):
    # This container's walrus rejects >1 sync-wait command per instruction
    # (CoreV3 setupSyncWait). Tile attaches one wait per producing logical
    # processor to a single instruction; hoist the excess onto same-engine
    # Drain carriers inserted immediately before it.
    for fn in nc.m.functions:
        for bb in fn.blocks:
            out = []
            changed = False
            for inst in bb.instructions:
                si = inst.sync_info
                waits = list(si.on_wait) if si and si.on_wait else []
                if len(waits) > max_waits:
                    rest = waits[max_waits:]
                    for j in range(0, len(rest), max_waits):
                        carrier = mybir.InstDrain(
                            name=nc.get_next_instruction_name(), ins=[], outs=[]
                        )
                        carrier.engine = inst.engine
                        carrier.sync_info = mybir.SyncInfo(
                            on_wait=rest[j : j + max_waits], on_update=[]
                        )
                        out.append(carrier)
                    si.on_wait = waits[:max_waits]
                    changed = True
                out.append(inst)
            if changed:
                bb.instructions = out


def _build(split_waits=True):
    nc = bass.Bass()
    x = nc.declare_dram_parameter("x", [IN_ROWS, WIDTH], F32R, isOutput=False)
    bands = nc.declare_dram_parameter("bands", [128, 3 * 128], F32R, isOutput=False)
    bands4 = nc.declare_dram_parameter("bands4", [32, 8], F32R, isOutput=False)
    bias = nc.declare_dram_parameter("bias", [128, 1], F32, isOutput=False)
    y = nc.declare_dram_parameter("y", [RPC, OW], F32, isOutput=True)

    ident = mybir.ActivationFunctionType.Identity

    with TileContext(nc) as tc:
        with (
            tc.tile_pool(name="const", bufs=1) as cpool,
            tc.tile_pool(name="xin", bufs=3) as xpool,
            tc.tile_pool(name="stage", bufs=3) as spool,
            tc.tile_pool(name="psum", bufs=8, space="PSUM") as ppool,
        ):
            band_f = cpool.tile([128, 3 * 128], F32R)
            nc.gpsimd.dma_start(out=band_f[:], in_=bands[:])
            band4_f = cpool.tile([32, 8], F32R)
            nc.gpsimd.dma_start(out=band4_f[:], in_=bands4[:])
            bias_t = cpool.tile([128, 1], F32)
            nc.gpsimd.dma_start(out=bias_t[:], in_=bias[:])
            stage4 = cpool.tile([8, WIDTH], F32)
            # prime the ACT function table (1.3us, once) long before the
            # endgame drains need the scalar engine
            prime_t = cpool.tile([128, 1], F32)
            nc.scalar.activation(prime_t[:1, :], bias_t[:1, :], ident,
                                 bias=bias_t[:1, :], scale=1.0)

            def drain(ct, dst, src, npart, split):
                # endgame: alternate engines so neither gates the PE
                if split and ct % 2 == 0:
                    nc.scalar.activation(dst, src, ident,
                                         bias=bias_t[:npart, :], scale=1.0)
                else:
                    nc.vector.tensor_scalar_add(dst, src, bias_t[:npart, :])

            # scalar store-issues are LAGGED by one group: emitted only
            # after the next group's drains, so their drain-sems are already
            # satisfied and the scalar queue (which also carries half the
            # drains) never head-of-line blocks
            pending = []

            def flush_pending():
                while pending:
                    dst, srcap = pending.pop(0)
                    nc.scalar.dma_start(out=dst, in_=srcap)

            def full_strip(s, xt, split_drains, last):
                r0 = 126 * s
                for g in range(2):
                    stage = spool.tile([128, 4096], F32, tag="st")
                    for j in range(8):
                        ct = g * 8 + j
                        c0 = ct * 512
                        n = 512 if ct < N_COL_TILES - 1 else 510
                        ps = ppool.tile([128, 512], F32, tag="ps")
                        for dj in range(KW):
                            nc.tensor.matmul(
                                ps[:126, :n],
                                band_f[:, dj * 128 : dj * 128 + 126],
                                xt[:, c0 + dj : c0 + dj + n],
                                start=(dj == 0),
                                stop=(dj == KW - 1),
                            )
                        drain(ct, stage[:126, j * 512 : j * 512 + n],
                              ps[:126, :n], 126, split_drains)
                        if last and g == 1 and j in (3, 5, 7):
                            # drip out the final stage in 1M/512K/512K pieces
                            # so the very last store is small
                            lo = {3: 0, 5: 2048, 7: 3072}[j]
                            hi = {3: 2048, 5: 3072, 7: 4094}[j]
                            nc.scalar.dma_start(
                                out=y[r0 : r0 + 126, 4096 + lo : 4096 + hi],
                                in_=stage[:126, lo:hi],
                            )
                    flush_pending()
                    if not (last and g == 1):
                        gw = 4096 if g == 0 else 4094
                        if last and g == 0:
                            # strip 3's g0 store rides the by-then-quiet SP
                            # ring, immediately (nothing queues behind it)
                            nc.sync.dma_start(
                                out=y[r0 : r0 + 126, 0:gw],
                                in_=stage[:126, :gw],
                            )
                        else:
                            pending.append((
                                y[r0 : r0 + 126, g * 4096 : g * 4096 + gw],
                                stage[:126, :gw],
                            ))

            xts = {}
            for s in range(3):
                r0 = 126 * s
                xt = xpool.tile([128, WIDTH], F32R, tag="xt")
                xts[s] = xt
                if s == 0:
                    # small first chunks so the first matmuls start early
                    # (chunk k must cover col tile k's 514-col window)
                    chunks = [(0, 640), (640, 1664), (1664, 3712), (3712, 8192)]
                else:
                    chunks = [(0, 4096), (4096, 8192)]
                for a, b in chunks:
                    nc.sync.dma_start(out=xt[:, a:b], in_=x[r0 : r0 + 128, a:b])
                full_strip(s, xt, split_drains=True, last=False)

            # strip-3 loads first: xt3 reuses s0's buffer (free earliest,
            # ~28us) so its 4MB lands by ~50us. The mini-strip tile follows
            # (s1's buffer); its im2col layout -- partition 3r + dj holds
            # X[504+r, dj:] -- means one K=30 matmul per col tile instead of
            # three K=10 ones (PE time is taxed 2x by the power throttle
            # while HBM is hot), and the dj-interleaved partitions spread
            # each 10-row load over 8 SBUF ports instead of 2-3.
            xt3 = xpool.tile([128, WIDTH], F32R, tag="xt")
            for a, b in [(0, 4096), (4096, 8192)]:
                nc.sync.dma_start(out=xt3[:, a:b], in_=x[378 : 378 + 128, a:b])
            x4 = xpool.tile([128, WIDTH], F32R, tag="xt")
            for dj in range(KW):
                nc.sync.dma_start(
                    out=x4[dj : 28 + dj + 1 : 3, 0 : WIDTH - dj],
                    in_=x[504:514, dj:WIDTH],
                )

            full_strip(3, xt3, split_drains=True, last=True)

            # mini-strip last: outputs 504..511 from input rows 504..513.
            # Its port-limited 8-partition store (SDMA slots 0/2 only) goes
            # out in quarters interleaved with the drains so the final
            # piece is tiny; quarters ride the sync ring, which is empty by
            # now, so they never block the scalar drain stream.
            for ct in range(N_COL_TILES):
                c0 = ct * 512
                n = 512 if ct < N_COL_TILES - 1 else 510
                ps = ppool.tile([128, 512], F32, tag="ps")
                nc.tensor.matmul(
                    ps[:8, :n],
                    band4_f[0:30, 0:8],
                    x4[0:30, c0 : c0 + n],
                    start=True,
                    stop=True,
                )
                drain(ct, stage4[:8, c0 : c0 + n], ps[:8, :n], 8, True)
                if ct % 4 == 3:
                    q0 = (ct - 3) * 512
                    q1 = min(ct * 512 + n, OW)
                    nc.sync.dma_start(
                        out=y[504:512, q0:q1], in_=stage4[:8, q0:q1]
                    )

    if split_waits:
        _split_multi_waits(nc)
    return nc


_NC_CACHE = None


def _get_nc():
    global _NC_CACHE
    if _NC_CACHE is None:
        _NC_CACHE = _build()
    return _NC_CACHE


def _make_host_inputs(X, W, b):
    X = np.ascontiguousarray(np.asarray(X, dtype=np.float32))
    W = np.asarray(W, dtype=np.float32)
    b = np.asarray(b, dtype=np.float32)

    bands = np.zeros((128, 3 * 128), dtype=np.float32)
    mm = np.arange(126)
    for dj in range(KW):
        for dk in range(KH):
            # B_dj[m+dk, m] = W[dk, dj] for every output row m
            bands[mm + dk, dj * 128 + mm] = W[dk, dj]
    # mini-strip im2col band: partition 3r + dj = input local row 504+r
    # shifted by dj cols; col m = output local row 504+m; B4[3r+dj, m] =
    # W[r-m, dj]
    bands4 = np.zeros((32, 8), dtype=np.float32)
    m8 = np.arange(8)
    for dj in range(KW):
        for dk in range(KH):
            bands4[3 * (m8 + dk) + dj, m8] = W[dk, dj]
    bias = np.full((128, 1), float(b[0]), dtype=np.float32)

    in_maps = []
    for i in range(N_CORES):
        r0 = i * RPC
        avail = min(IN_ROWS, H - r0)
        if avail == IN_ROWS:
            shard = X[r0 : r0 + IN_ROWS]
        else:
            shard = np.zeros((IN_ROWS, WIDTH), dtype=np.float32)
            shard[:avail] = X[r0 : r0 + avail]
        in_maps.append({"x": shard, "bands": bands, "bands4": bands4, "bias": bias})
    return in_maps


def _assemble(results):
    out = np.empty((OH, OW), dtype=np.float32)
    for i in range(N_CORES):
        r0 = i * RPC
        take = min(RPC, OH - r0)
        out[r0 : r0 + take] = results[i]["y"][:take]
    return out


def run(X, W, b, trace=False):
    nc = _get_nc()
    in_maps = _make_host_inputs(X, W, b)
    res = run_bass_kernel_spmd(nc, in_maps, list(range(N_CORES)), trace=trace)
    return _assemble(res.results), res


def kernel(X, W, b):
    out, _ = run(X, W, b)
    return out



# revision 2
# speedup vs baseline: 1.5964x; 1.5964x over previous
"""3x3 valid conv (single channel) on 8 TRN2 NeuronCores.

Strategy: shard X row-wise (512 output rows/core), fp16 end-to-end.
The problem is memory-bound at fp32 (34MB/core); converting X to fp16 on
host and storing y as fp16 (upcast on host) halves HBM traffic to
~17MB/core. fp16 matmul runs at 1 row/cycle (same as fp32r) with exact
f32 PSUM accumulation, so the only precision cost is the input/output
rounding: ~6e-4 relative — far inside the 2e-2 gate.

Per core, five row strips: four full strips load 128 input rows each
(rows 126s..126s+127, re-reading the 2-row halo from HBM) and produce
126 output rows via 3 banded matmuls per 512-col PSUM tile; a mini-strip
loads rows 504..513 (10 rows, im2col-skewed so one K=30 matmul per col
tile) and produces the remaining 8 output rows.

Scheduling: loads ride the SP HWDGE ring in chunks (first chunk small so
the first matmul starts right after the engine preamble); stores ride
the ACT ring so loads can't head-of-line-block them. PSUM drains (bias
add + f32->fp16 cast) alternate vector/scalar so a single engine's
drain rate can't gate the PE's ~700ns/tile supply.
"""

import sys

sys.path.insert(0, "/opt/trn_rl_repo")

import numpy as np
from concourse import bass, mybir
from concourse.bass_utils import run_bass_kernel_spmd
from concourse.tile import TileContext

F32 = mybir.dt.float32
F16 = mybir.dt.float16

H, WIDTH = 4096, 8192
KH, KW = 3, 3
OH, OW = H - KH + 1, WIDTH - KW + 1
N_CORES = 8
RPC = H // N_CORES          # 512 output rows produced per core
IN_ROWS = RPC + KH - 1      # 514 input rows per core (2-row halo)
N_COL_TILES = 16            # 15 x 512 + 1 x 510 = 8190


def _split_multi_waits(nc, max_waits=1):
    # This container's walrus rejects >1 sync-wait command per instruction
    # (CoreV3 setupSyncWait). Tile attaches one wait per producing logical
    # processor to a single instruction; hoist the excess onto same-engine
    # Drain carriers inserted immediately before it.
    for fn in nc.m.functions:
        for bb in fn.blocks:
            out = []
            changed = False
            for inst in bb.instructions:
                si = inst.sync_info
                waits = list(si.on_wait) if si and si.on_wait else []
                if len(waits) > max_waits:
                    rest = waits[max_waits:]
                    for j in range(0, len(rest), max_waits):
                        carrier = mybir.InstDrain(
                            name=nc.get_next_instruction_name(), ins=[], outs=[]
                        )
                        carrier.engine = inst.engine
                        carrier.sync_info = mybir.SyncInfo(
                            on_wait=rest[j : j + max_waits], on_update=[]
                        )
                        out.append(carrier)
                    si.on_wait = waits[:max_waits]
                    changed = True
                out.append(inst)
            if changed:
                bb.instructions = out


def _build(split_waits=True):
    nc = bass.Bass()
    x = nc.declare_dram_parameter("x", [IN_ROWS, WIDTH], F16, isOutput=False)
    bands = nc.declare_dram_parameter("bands", [128, 3 * 128], F16, isOutput=False)
    bands4 = nc.declare_dram_parameter("bands4", [32, 8], F16, isOutput=False)
    bias = nc.declare_dram_parameter("bias", [128, 1], F32, isOutput=False)
    y = nc.declare_dram_parameter("y", [RPC, OW], F16, isOutput=True)

    ident = mybir.ActivationFunctionType.Identity

    with TileContext(nc) as tc:
        with (
            tc.tile_pool(name="const", bufs=1) as cpool,
            tc.tile_pool(name="xin", bufs=3) as xpool,
            tc.tile_pool(name="stage", bufs=3) as spool,
            tc.tile_pool(name="psum", bufs=8, space="PSUM") as ppool,
        ):
            band_f = cpool.tile([128, 3 * 128], F16)
            nc.gpsimd.dma_start(out=band_f[:], in_=bands[:])
            band4_f = cpool.tile([32, 8], F16)
            nc.gpsimd.dma_start(out=band4_f[:], in_=bands4[:])
            bias_t = cpool.tile([128, 1], F32)
            nc.gpsimd.dma_start(out=bias_t[:], in_=bias[:])
            stage4 = cpool.tile([8, WIDTH], F16)
            # prime the ACT function table (1.3us, once) long before the
            # endgame drains need the scalar engine
            prime_t = cpool.tile([128, 1], F32)
            nc.scalar.activation(prime_t[:1, :], bias_t[:1, :], ident,
                                 bias=bias_t[:1, :], scale=1.0)

            def drain(ct, dst, src, npart, split):
                # alternate engines so neither gates the PE
                if split and ct % 2 == 0:
                    nc.scalar.activation(dst, src, ident,
                                         bias=bias_t[:npart, :], scale=1.0)
                else:
                    nc.vector.tensor_scalar_add(dst, src, bias_t[:npart, :])

            # scalar store-issues are LAGGED by one group: emitted only
            # after the next group's drains, so their drain-sems are already
            # satisfied and the scalar queue (which also carries half the
            # drains) never head-of-line blocks
            pending = []

            def flush_pending():
                while pending:
                    dst, srcap = pending.pop(0)
                    nc.scalar.dma_start(out=dst, in_=srcap)

            def full_strip(s, xt, split_drains, last):
                r0 = 126 * s
                for g in range(2):
                    stage = spool.tile([128, 4096], F16, tag="st")
                    for j in range(8):
                        ct = g * 8 + j
                        c0 = ct * 512
                        n = 512 if ct < N_COL_TILES - 1 else 510
                        ps = ppool.tile([128, 512], F32, tag="ps")
                        for dj in range(KW):
                            nc.tensor.matmul(
                                ps[:126, :n],
                                band_f[:, dj * 128 : dj * 128 + 126],
                                xt[:, c0 + dj : c0 + dj + n],
                                start=(dj == 0),
                                stop=(dj == KW - 1),
                            )
                        drain(ct, stage[:126, j * 512 : j * 512 + n],
                              ps[:126, :n], 126, split_drains)
                        if last and g == 1 and j in (3, 5, 7):
                            # drip out the final stage in pieces so the very
                            # last store is small
                            lo = {3: 0, 5: 2048, 7: 3072}[j]
                            hi = {3: 2048, 5: 3072, 7: 4094}[j]
                            nc.scalar.dma_start(
                                out=y[r0 : r0 + 126, 4096 + lo : 4096 + hi],
                                in_=stage[:126, lo:hi],
                            )
                    flush_pending()
                    if not (last and g == 1):
                        gw = 4096 if g == 0 else 4094
                        if last and g == 0:
                            # strip 3's g0 store rides the by-then-quiet SP
                            # ring, immediately (nothing queues behind it)
                            nc.sync.dma_start(
                                out=y[r0 : r0 + 126, 0:gw],
                                in_=stage[:126, :gw],
                            )
                        else:
                            pending.append((
                                y[r0 : r0 + 126, g * 4096 : g * 4096 + gw],
                                stage[:126, :gw],
                            ))

            xts = {}
            for s in range(3):
                r0 = 126 * s
                xt = xpool.tile([128, WIDTH], F16, tag="xt")
                xts[s] = xt
                if s == 0:
                    # small first chunks so the first matmuls start early
                    # (chunk k must cover col tile k's 514-col window)
                    chunks = [(0, 640), (640, 1664), (1664, 3712), (3712, 8192)]
                else:
                    chunks = [(0, 4096), (4096, 8192)]
                for a, b in chunks:
                    nc.sync.dma_start(out=xt[:, a:b], in_=x[r0 : r0 + 128, a:b])
                full_strip(s, xt, split_drains=True, last=False)

            # strip-3 loads first: xt3 reuses s0's buffer (free earliest)
            # so its 2MB lands early. The mini-strip tile follows (s1's
            # buffer); its im2col layout -- partition 3r + dj holds
            # X[504+r, dj:] -- means one K=30 matmul per col tile instead
            # of three K=10 ones, and the dj-interleaved partitions spread
            # each 10-row load over 8 SBUF ports instead of 2-3.
            xt3 = xpool.tile([128, WIDTH], F16, tag="xt")
            for a, b in [(0, 4096), (4096, 8192)]:
                nc.sync.dma_start(out=xt3[:, a:b], in_=x[378 : 378 + 128, a:b])
            x4 = xpool.tile([128, WIDTH], F16, tag="xt")
            for dj in range(KW):
                nc.sync.dma_start(
                    out=x4[dj : 28 + dj + 1 : 3, 0 : WIDTH - dj],
                    in_=x[504:514, dj:WIDTH],
                )

            full_strip(3, xt3, split_drains=True, last=True)

            # mini-strip last: outputs 504..511 from input rows 504..513.
            # Its port-limited 8-partition store goes out in quarters
            # interleaved with the drains so the final piece is tiny;
            # quarters ride the sync ring, which is empty by now.
            for ct in range(N_COL_TILES):
                c0 = ct * 512
                n = 512 if ct < N_COL_TILES - 1 else 510
                ps = ppool.tile([128, 512], F32, tag="ps")
                nc.tensor.matmul(
                    ps[:8, :n],
                    band4_f[0:30, 0:8],
                    x4[0:30, c0 : c0 + n],
                    start=True,
                    stop=True,
                )
                drain(ct, stage4[:8, c0 : c0 + n], ps[:8, :n], 8, True)
                if ct % 4 == 3:
                    q0 = (ct - 3) * 512
                    q1 = min(ct * 512 + n, OW)
                    nc.sync.dma_start(
                        out=y[504:512, q0:q1], in_=stage4[:8, q0:q1]
                    )

    if split_waits:
        _split_multi_waits(nc)
    return nc


_NC_CACHE = None


def _get_nc():
    global _NC_CACHE
    if _NC_CACHE is None:
        _NC_CACHE = _build()
    return _NC_CACHE


def _make_host_inputs(X, W, b):
    Xh = np.ascontiguousarray(np.asarray(X, dtype=np.float32).astype(np.float16))
    W = np.asarray(W, dtype=np.float32)
    b = np.asarray(b, dtype=np.float32)

    bands = np.zeros((128, 3 * 128), dtype=np.float16)
    mm = np.arange(126)
    for dj in range(KW):
        for dk in range(KH):
            # B_dj[m+dk, m] = W[dk, dj] for every output row m
            bands[mm + dk, dj * 128 + mm] = W[dk, dj]
    # mini-strip im2col band: partition 3r + dj = input local row 504+r
    # shifted by dj cols; col m = output local row 504+m; B4[3r+dj, m] =
    # W[r-m, dj]
    bands4 = np.zeros((32, 8), dtype=np.float16)
    m8 = np.arange(8)
    for dj in range(KW):
        for dk in range(KH):
            bands4[3 * (m8 + dk) + dj, m8] = W[dk, dj]
    bias = np.full((128, 1), float(b[0]), dtype=np.float32)

    in_maps = []
    for i in range(N_CORES):
        r0 = i * RPC
        avail = min(IN_ROWS, H - r0)
        if avail == IN_ROWS:
            shard = Xh[r0 : r0 + IN_ROWS]
        else:
            shard = np.zeros((IN_ROWS, WIDTH), dtype=np.float16)
            shard[:avail] = Xh[r0 : r0 + avail]
        in_maps.append({"x": shard, "bands": bands, "bands4": bands4, "bias": bias})
    return in_maps


def _assemble(results):
    out = np.empty((OH, OW), dtype=np.float32)
    for i in range(N_CORES):
        r0 = i * RPC
        take = min(RPC, OH - r0)
        out[r0 : r0 + take] = results[i]["y"][:take].astype(np.float32)
    return out


def run(X, W, b, trace=False):
    nc = _get_nc()
    in_maps = _make_host_inputs(X, W, b)
    res = run_bass_kernel_spmd(nc, in_maps, list(range(N_CORES)), trace=trace)
    return _assemble(res.results), res


def kernel(X, W, b):
    out, _ = run(X, W, b)
    return out
